# revision 10
# baseline (speedup 1.0000x reference)
"""APPNP (GNN message passing) distributed Bass kernel for 8 TRN2 NeuronCores.

Strategy (dst-sharded, SPMD):
- Host: bucket nodes by padded in-degree (multiples of 4), deal round-robin
  to 8 cores / 128 partitions (uniform layout across cores), build per-core
  gather index arrays over the padded dst-sorted edge-slot layout.
- Device, per core: bf16 MLP (feature-major, last layer flipped), degree
  norms, then 10 steps of:
    scale h by norm_src -> AllGather (bf16, 8-nodes-per-256B-row table) ->
    chunked dma_gather of full 256B rows (idx = src>>3, int16) ->
    DVE lane-extract via precomputed one-hot masks ->
    per-bucket strided tensor_reduce segmented sum -> h update.
- Output unsharded on host.
"""
import os
import sys
import numpy as np

for _p in ("/opt/trn_rl_repo", "/opt/pypackages"):
    if _p not in sys.path:
        sys.path.append(_p)

from concourse import bass, bacc, tile, mybir
from concourse.bass_utils import run_bass_kernel_spmd

ALPHA = 0.1
K_STEPS = int(os.environ.get("APPNP_K", "10"))
NCORES = 8
P = 128
NCOL = 512            # MLP column chunk
LANES = 8             # nodes per 256B table row
GCHUNK = 1024         # gather descriptors per SWDGE instruction (ring cap)

fp32 = mybir.dt.float32
bf16 = mybir.dt.bfloat16
i16 = mybir.dt.int16

_ml_dtypes = None
_IOTA8 = None


def _bf16():
    global _ml_dtypes, _IOTA8
    if _ml_dtypes is None:
        import ml_dtypes
        _ml_dtypes = ml_dtypes
        _IOTA8 = np.broadcast_to(
            np.arange(LANES, dtype=ml_dtypes.bfloat16), (P, LANES)).copy()
    return _ml_dtypes.bfloat16


# ---------------------------------------------------------------- planning --

class _Plan:
    pass


def _make_plan(src, dst, N):
    E = src.shape[0]
    deg_in = np.bincount(dst, minlength=N).astype(np.int64)
    deg_out = np.bincount(src, minlength=N).astype(np.int64)

    Dn = np.maximum(4, ((deg_in + 3) // 4) * 4)
    bvals = np.unique(Dn)

    core_of = np.empty(N, np.int32)
    part_of = np.empty(N, np.int32)
    jpos = np.empty(N, np.int64)
    slot0 = np.empty(N, np.int64)

    joff = 0
    soff = 0
    bucket_meta = []
    for D in bvals:
        nodes_b = np.nonzero(Dn == D)[0]
        cnt = len(nodes_b)
        cbmax = (cnt + NCORES - 1) // NCORES
        m = (cbmax + P - 1) // P
        i = np.arange(cnt)
        c = i % NCORES
        r = i // NCORES
        core_of[nodes_b] = c
        part_of[nodes_b] = r % P
        jpos[nodes_b] = joff + r // P
        slot0[nodes_b] = soff + (r // P) * D
        bucket_meta.append((int(D), int(m), int(joff), int(soff)))
        joff += m
        soff += m * D

    if joff % 4:
        pad = 4 - joff % 4
        bucket_meta.append((4, pad, joff, soff))
        joff += pad
        soff += 4 * pad
    # pad slot columns to a multiple of GCHUNK//P without breaking 4 | n:
    # soff is a multiple of 4, so the residue r is 0 or 4; add 4 nodes x D=1.
    spp = GCHUNK // P  # slots per partition per chunk
    r = (spp - soff % spp) % spp
    if r:
        assert r % 4 == 0
        bucket_meta.append((r // 4, 4, joff, soff))
        joff += 4
        soff += r

    n = joff
    s_tot = soff
    ROWS = P * n
    R_TOT = NCORES * ROWS
    assert R_TOT % LANES == 0
    assert R_TOT // LANES <= 32767, "int16 gather index limit"

    gid = core_of.astype(np.int64) * ROWS + part_of.astype(np.int64) * n + jpos

    occ = np.zeros((NCORES, P, n), dtype=bool)
    occ[core_of, part_of, jpos] = True
    free = np.nonzero(~occ.reshape(-1))[0]
    assert len(free) > 0
    dummy_gid = int(free[0])

    order = np.argsort(dst, kind="stable")
    starts = np.zeros(N + 1, np.int64)
    np.cumsum(deg_in, out=starts[1:])
    rank = np.arange(E, dtype=np.int64) - starts[dst[order]]
    d_ord = dst[order]
    s_ord = src[order]

    gsrc = np.full((NCORES, P, s_tot), dummy_gid, dtype=np.int64)
    gsrc[core_of[d_ord], part_of[d_ord], slot0[d_ord] + rank] = gid[s_ord]

    # gather row index (int16) and lane value per slot (mask built on device)
    idx_row = (gsrc >> 3).astype(np.int16)          # [NCORES, P, s_tot]
    lanev = (gsrc & 7).astype(_bf16())              # [NCORES, P, s_tot]

    deg_in_t = np.zeros((NCORES, P, n), np.float32)
    deg_out_t = np.zeros((NCORES, P, n), np.float32)
    deg_in_t[core_of, part_of, jpos] = deg_in
    deg_out_t[core_of, part_of, jpos] = deg_out

    plan = _Plan()
    plan.N, plan.E = N, E
    plan.n, plan.s_tot, plan.ROWS, plan.R_TOT = n, s_tot, ROWS, R_TOT
    plan.bucket_meta = bucket_meta
    plan.core_of, plan.part_of, plan.jpos = core_of, part_of, jpos
    plan.idx_row, plan.lanev = idx_row, lanev
    plan.deg_in_t, plan.deg_out_t = deg_in_t, deg_out_t
    return plan


def _wrap_idx_chunks(idx_row_core, s_tot):
    """Per gather chunk: [128, spp] slot block -> int16 wrapped [128, GCHUNK//16].
    Gather position i (p=i%128, col=i//128) must map to slot (p, cs+col):
    flat list f[i] = idx_row[i % 128, cs + i // 128]."""
    spp = GCHUNK // P
    nch = s_tot // spp
    out = np.empty((P, nch, GCHUNK // 16), np.int16)
    for ch in range(nch):
        blk = idx_row_core[:, ch * spp:(ch + 1) * spp]     # [128, spp]
        flat = blk.T.reshape(-1)                           # i = col*128 + p
        w = flat.reshape(GCHUNK // 16, 16).T               # [16, GCHUNK//16]
        out[:, ch, :] = np.tile(w, (8, 1))
    return out                                             # [128, nch, GCHUNK//16]


# ------------------------------------------------------------------ builder --

def _raw_dma_gather(nc, out_ap, in_ap, idxs_ap, num_idxs, elem_size, elem_step,
                    queue_num=0):
    g = nc.gpsimd
    stride_bytes = elem_step * mybir.dt.size(in_ap.dtype)
    assert stride_bytes % 256 == 0
    _in_ap = g.lower_ap_dma(in_ap, for_custom_bir_dma=True)
    _idxs_ap = g.lower_ap(idxs_ap)
    _out_ap = g.lower_ap(out_ap)
    return g.add_instruction(
        mybir.InstDMAGatherAnt(
            name=nc.get_next_instruction_name(),
            ins=[*_in_ap, _idxs_ap, g.lower_val_access(g.to_reg(num_idxs))],
            outs=[_out_ap],
            transpose=False,
            num_idxs=num_idxs,
            elem_size=elem_size,
            stride_bytes_256=stride_bytes // 256,
            gen_mode=0,
            single_packet=True,
            queue_num=queue_num,
            sbuf_tokens_per_rank=0,
            sbuf_free_dim_per_rank=0,
            sbuf_free_dim_pad_per_rank=0,
            sbuf_byte_offset=0,
        )
    )


def _build(plan, F, H, C):
    n, s_tot, ROWS, R_TOT = plan.n, plan.s_tot, plan.ROWS, plan.R_TOT
    assert ROWS % NCOL == 0
    ncol_chunks = ROWS // NCOL
    spp = GCHUNK // P                 # slot columns per gather chunk
    nch = s_tot // spp                # gather chunks per step
    TROWS = R_TOT // LANES            # 256B table rows

    nc = bacc.Bacc("TRN2", target_bir_lowering=False, debug=False,
                   num_devices=NCORES, num_swdge_queues=4)

    ftT = nc.declare_dram_parameter("featT", [F, ROWS], fp32, isOutput=False)
    w0 = nc.declare_dram_parameter("W0", [F, H], fp32, isOutput=False)
    b0 = nc.declare_dram_parameter("b0", [H, 1], fp32, isOutput=False)
    w1 = nc.declare_dram_parameter("W1", [H, H], fp32, isOutput=False)
    b1 = nc.declare_dram_parameter("b1", [H, 1], fp32, isOutput=False)
    w2 = nc.declare_dram_parameter("W2", [H, C], fp32, isOutput=False)
    b2r = nc.declare_dram_parameter("b2r", [P, C], fp32, isOutput=False)
    idxp = nc.declare_dram_parameter("idx", [P, nch, GCHUNK // 16], i16,
                                     isOutput=False)
    lvp = nc.declare_dram_parameter("lanev", [P, s_tot], bf16, isOutput=False)
    iop = nc.declare_dram_parameter("iota8", [P, LANES], bf16, isOutput=False)
    degi = nc.declare_dram_parameter("degi", [P, n], fp32, isOutput=False)
    dego = nc.declare_dram_parameter("dego", [P, n], fp32, isOutput=False)
    outp = nc.declare_dram_parameter("out", [P, n, C], fp32, isOutput=True)

    cc_in = nc.dram_tensor("cc_in", [P, n, C], bf16)
    cc_out = nc.dram_tensor("cc_out", [TROWS, LANES * C], bf16)

    with tile.TileContext(nc) as tc:
        with tc.tile_pool(name="persist", bufs=1) as pers, \
             tc.tile_pool(name="work", bufs=3) as work, \
             tc.tile_pool(name="gbuf", bufs=2) as gbuf, \
             tc.tile_pool(name="selp", bufs=2) as selp, \
             tc.tile_pool(name="psum", bufs=2, space="PSUM") as psum, \
             tc.tile_pool(name="psum3", bufs=2, space="PSUM") as psum3p:

            w0a = pers.tile([P, H], bf16, tag="w0a")
            w0b = pers.tile([P, H], bf16, tag="w0b")
            w1t = pers.tile([P, H], bf16, tag="w1t")
            w2t = pers.tile([P, C], bf16, tag="w2t")
            b0t = pers.tile([H, 1], fp32, tag="b0t")
            b1t = pers.tile([H, 1], fp32, tag="b1t")
            b2t = pers.tile([P, C], fp32, tag="b2t")
            idx_sb = pers.tile([P, nch, GCHUNK // 16], i16, tag="idx")
            lv_sb = pers.tile([P, s_tot], bf16, tag="lanev")
            io_sb = pers.tile([P, LANES], bf16, tag="iota8")
            nsrc = pers.tile([P, n], fp32, tag="nsrc")
            ndst9 = pers.tile([P, n], fp32, tag="ndst9")
            h = pers.tile([P, n, C], fp32, tag="h")
            h0s = pers.tile([P, n, C], fp32, tag="h0s")
            agg = pers.tile([P, n, C], fp32, tag="agg")
            hs = pers.tile([P, n, C], bf16, tag="hs")
            msgs = pers.tile([P, s_tot, C], bf16, tag="msgs")
            h2T = pers.tile([P, ROWS], bf16, tag="h2T")

            for wt, src_ap in ((w0a, w0[0:P, :]), (w0b, w0[P:2 * P, :]),
                               (w1t, w1[:, :]), (w2t, w2[:, :])):
                tmp = work.tile(list(src_ap.shape), fp32, tag="wload")
                nc.sync.dma_start(out=tmp[:], in_=src_ap)
                nc.vector.tensor_copy(out=wt[:], in_=tmp[:])
            nc.sync.dma_start(out=b0t[:], in_=b0[:, :])
            nc.sync.dma_start(out=b1t[:], in_=b1[:, :])
            nc.sync.dma_start(out=b2t[:], in_=b2r[:, :])
            nc.sync.dma_start(out=idx_sb[:], in_=idxp[:, :, :])
            nc.sync.dma_start(out=lv_sb[:], in_=lvp[:, :])
            nc.sync.dma_start(out=io_sb[:], in_=iop[:, :])

            dtmp = work.tile([P, n], fp32, tag="deg")
            mask = work.tile([P, n], fp32, tag="mask")
            for deg_p, out_t, scale in ((dego, nsrc, 1.0),
                                        (degi, ndst9, 1.0 - ALPHA)):
                dsb = work.tile([P, n], fp32, tag="degload")
                nc.sync.dma_start(out=dsb[:], in_=deg_p[:, :])
                nc.vector.tensor_scalar(out=dtmp[:], in0=dsb[:], scalar1=1.0,
                                        scalar2=None, op0=mybir.AluOpType.max)
                nc.scalar.sqrt(out=dtmp[:], in_=dtmp[:])
                nc.vector.reciprocal(out=dtmp[:], in_=dtmp[:])
                nc.vector.tensor_scalar(out=mask[:], in0=dsb[:], scalar1=0.0,
                                        scalar2=None, op0=mybir.AluOpType.is_gt)
                if scale != 1.0:
                    nc.vector.tensor_scalar_mul(out=mask[:], in0=mask[:],
                                                scalar1=scale)
                nc.vector.tensor_tensor(out=out_t[:], in0=dtmp[:], in1=mask[:],
                                        op=mybir.AluOpType.mult)

            # ---- MLP
            for ch in range(ncol_chunks):
                cs = ch * NCOL
                xa = work.tile([P, NCOL], fp32, tag="xa")
                xb = work.tile([P, NCOL], fp32, tag="xb")
                nc.sync.dma_start(out=xa[:], in_=ftT[0:P, cs:cs + NCOL])
                nc.sync.dma_start(out=xb[:], in_=ftT[P:2 * P, cs:cs + NCOL])
                xab = work.tile([P, NCOL], bf16, tag="xab")
                xbb = work.tile([P, NCOL], bf16, tag="xbb")
                nc.vector.tensor_copy(out=xab[:], in_=xa[:])
                nc.vector.tensor_copy(out=xbb[:], in_=xb[:])
                ps1 = psum.tile([H, NCOL], fp32, tag="ps1")
                nc.tensor.matmul(ps1[:], w0a[:], xab[:], start=True, stop=False)
                nc.tensor.matmul(ps1[:], w0b[:], xbb[:], start=False, stop=True)
                h1 = work.tile([H, NCOL], bf16, tag="h1")
                nc.scalar.activation(out=h1[:], in_=ps1[:],
                                     func=mybir.ActivationFunctionType.Relu,
                                     bias=b0t[:, :1])
                ps2 = psum.tile([H, NCOL], fp32, tag="ps2")
                nc.tensor.matmul(ps2[:], w1t[:], h1[:], start=True, stop=True)
                nc.scalar.activation(out=h2T[:, cs:cs + NCOL], in_=ps2[:],
                                     func=mybir.ActivationFunctionType.Relu,
                                     bias=b1t[:, :1])
            for j in range(n):
                ps3 = psum3p.tile([P, C], fp32, tag="ps3")
                nc.tensor.matmul(ps3[:], h2T[:, j * P:(j + 1) * P], w2t[:],
                                 start=True, stop=True)
                nc.vector.tensor_tensor(out=h[:, j, :], in0=ps3[:], in1=b2t[:],
                                        op=mybir.AluOpType.add)

            nc.vector.tensor_scalar_mul(out=h0s[:], in0=h[:], scalar1=ALPHA)

            nsrc_b = nsrc[:].unsqueeze(2).to_broadcast([P, n, C])
            ndst9_b = ndst9[:].unsqueeze(2).to_broadcast([P, n, C])

            # ---- propagation
            for k in range(K_STEPS):
                nc.vector.tensor_tensor(out=hs[:], in0=h[:], in1=nsrc_b,
                                        op=mybir.AluOpType.mult)
                nc.sync.dma_start(out=cc_in[:, :, :], in_=hs[:])
                nc.gpsimd.collective_compute(
                    "AllGather",
                    mybir.AluOpType.bypass,
                    replica_groups=[list(range(NCORES))],
                    ins=[cc_in.ap().opt()],
                    outs=[cc_out.ap().opt()],
                )
                for ch in range(nch):
                    c0 = ch * spp
                    wide = gbuf.tile([P, spp, LANES * C], bf16, tag=f"wide{ch % 4}")
                    _raw_dma_gather(nc, wide[:, :, :], cc_out[:, :],
                                    idx_sb[:, ch, :], GCHUNK, LANES * C,
                                    LANES * C, queue_num=ch % 4)
                    # lane extract: wide [p, spp, LANES, C] * mask -> sum lanes
                    wv = wide[:].rearrange("p s (l c) -> p s l c", l=LANES)
                    mkc = selp.tile([P, spp, LANES], bf16, tag="mkc")
                    nc.vector.tensor_tensor(
                        out=mkc[:],
                        in0=lv_sb[:, c0:c0 + spp].unsqueeze(2).to_broadcast(
                            [P, spp, LANES]),
                        in1=io_sb[:].unsqueeze(1).to_broadcast([P, spp, LANES]),
                        op=mybir.AluOpType.is_equal)
                    mk = mkc[:].unsqueeze(3).to_broadcast([P, spp, LANES, C])
                    sel = selp.tile([P, spp, LANES, C], bf16, tag="sel")
                    nc.vector.tensor_tensor(out=sel[:], in0=wv, in1=mk,
                                            op=mybir.AluOpType.mult)
                    sv = sel[:].rearrange("p s l c -> p s c l")
                    # one-hot lane sum: 7 of 8 terms are exactly zero, so a
                    # bf16 destination loses nothing
                    with nc.allow_low_precision(reason="one-hot lane select"):
                        nc.vector.tensor_reduce(out=msgs[:, c0:c0 + spp, :],
                                                in_=sv,
                                                axis=mybir.AxisListType.X,
                                                op=mybir.AluOpType.add)
                for (D, m, joff, soff) in plan.bucket_meta:
                    view = msgs[:, soff:soff + m * D, :].rearrange(
                        "p (m d) c -> p m c d", m=m)
                    nc.vector.tensor_reduce(
                        out=agg[:, joff:joff + m, :], in_=view,
                        axis=mybir.AxisListType.X, op=mybir.AluOpType.add)
                nc.vector.tensor_tensor(out=h[:], in0=agg[:], in1=ndst9_b,
                                        op=mybir.AluOpType.mult)
                nc.vector.tensor_tensor(out=h[:], in0=h[:], in1=h0s[:],
                                        op=mybir.AluOpType.add)

            nc.sync.dma_start(out=outp[:, :, :], in_=h[:])

    nc.compile()
    return nc


# ------------------------------------------------------------------- kernel --

def kernel(features, W0, b0, W1, b1, W2, b2, src, dst):
    features = np.asarray(features, dtype=np.float32)
    W0 = np.asarray(W0, dtype=np.float32)
    b0 = np.asarray(b0, dtype=np.float32)
    W1 = np.asarray(W1, dtype=np.float32)
    b1 = np.asarray(b1, dtype=np.float32)
    W2 = np.asarray(W2, dtype=np.float32)
    b2 = np.asarray(b2, dtype=np.float32)
    src = np.asarray(src, dtype=np.int32)
    dst = np.asarray(dst, dtype=np.int32)

    N, F = features.shape
    H = W0.shape[1]
    C = W2.shape[1]

    plan = _make_plan(src, dst, N)
    nc = _build(plan, F, H, C)

    col = plan.jpos * P + plan.part_of
    b2rep = np.broadcast_to(b2.reshape(1, C), (P, C)).astype(np.float32).copy()
    in_maps = []
    for c in range(NCORES):
        sel = plan.core_of == c
        ft = np.zeros((F, plan.ROWS), np.float32)
        ft[:, col[sel]] = features[sel].T
        in_maps.append({
            "featT": ft,
            "W0": W0, "b0": b0.reshape(H, 1),
            "W1": W1, "b1": b1.reshape(H, 1),
            "W2": W2, "b2r": b2rep,
            "idx": _wrap_idx_chunks(plan.idx_row[c], plan.s_tot),
            "lanev": plan.lanev[c],
            "iota8": _IOTA8,
            "degi": plan.deg_in_t[c],
            "dego": plan.deg_out_t[c],
        })

    res = run_bass_kernel_spmd(nc, in_maps, core_ids=list(range(NCORES)),
                               trace=False)

    arr = np.stack([res.results[c]["out"] for c in range(NCORES)])
    arr = arr.reshape(NCORES, P, plan.n, C)
    out = np.empty((N, C), np.float32)
    out[:] = arr[plan.core_of, plan.part_of, plan.jpos]
    return out


# revision 12
# speedup vs baseline: 2.7773x; 2.7773x over previous
"""APPNP (GNN message passing) distributed Bass kernel for 8 TRN2 NeuronCores.

Strategy (dst-sharded, SPMD):
- Host: bucket nodes by padded in-degree (multiples of 4), deal round-robin
  to 8 cores / 128 partitions (uniform layout across cores), build per-core
  gather index arrays over the padded dst-sorted edge-slot layout.
- Device, per core: bf16 MLP (feature-major, last layer flipped), degree
  norms, then 10 steps of:
    scale h by norm_src -> AllGather (bf16, 8-nodes-per-256B-row table) ->
    chunked dma_gather of full 256B rows (idx = src>>3, int16) ->
    DVE lane-extract via precomputed one-hot masks ->
    per-bucket strided tensor_reduce segmented sum -> h update.
- Output unsharded on host.
"""
import os
import sys
import numpy as np

for _p in ("/opt/trn_rl_repo", "/opt/pypackages"):
    if _p not in sys.path:
        sys.path.append(_p)

from concourse import bass, bacc, tile, mybir
from concourse.bass_utils import run_bass_kernel_spmd

ALPHA = 0.1
K_STEPS = int(os.environ.get("APPNP_K", "10"))
NQ = int(os.environ.get("APPNP_NQ", "4"))
SKIP_CC = bool(int(os.environ.get("APPNP_SKIP_CC", "0")))
SKIP_EXTRACT = bool(int(os.environ.get("APPNP_SKIP_EXTRACT", "0")))
SKIP_GATHER = bool(int(os.environ.get("APPNP_SKIP_GATHER", "0")))
NCORES = 8
P = 128
NCOL = 512            # MLP column chunk
LANES = 8             # nodes per 256B table row
GCHUNK = 1024         # gather descriptors per SWDGE instruction (ring cap)

fp32 = mybir.dt.float32
bf16 = mybir.dt.bfloat16
i16 = mybir.dt.int16

_ml_dtypes = None
_IOTA8 = None


def _bf16():
    global _ml_dtypes, _IOTA8
    if _ml_dtypes is None:
        import ml_dtypes
        _ml_dtypes = ml_dtypes
        _IOTA8 = np.broadcast_to(
            np.arange(LANES, dtype=ml_dtypes.bfloat16), (P, LANES)).copy()
    return _ml_dtypes.bfloat16


# ---------------------------------------------------------------- planning --

class _Plan:
    pass


def _make_plan(src, dst, N):
    E = src.shape[0]
    deg_in = np.bincount(dst, minlength=N).astype(np.int64)
    deg_out = np.bincount(src, minlength=N).astype(np.int64)

    Dn = np.maximum(4, ((deg_in + 3) // 4) * 4)
    bvals = np.unique(Dn)

    core_of = np.empty(N, np.int32)
    part_of = np.empty(N, np.int32)
    jpos = np.empty(N, np.int64)
    slot0 = np.empty(N, np.int64)

    joff = 0
    soff = 0
    bucket_meta = []
    for D in bvals:
        nodes_b = np.nonzero(Dn == D)[0]
        cnt = len(nodes_b)
        cbmax = (cnt + NCORES - 1) // NCORES
        m = (cbmax + P - 1) // P
        i = np.arange(cnt)
        c = i % NCORES
        r = i // NCORES
        core_of[nodes_b] = c
        part_of[nodes_b] = r % P
        jpos[nodes_b] = joff + r // P
        slot0[nodes_b] = soff + (r // P) * D
        bucket_meta.append((int(D), int(m), int(joff), int(soff)))
        joff += m
        soff += m * D

    if joff % 4:
        pad = 4 - joff % 4
        bucket_meta.append((4, pad, joff, soff))
        joff += pad
        soff += 4 * pad
    # pad slot columns to a multiple of GCHUNK//P without breaking 4 | n:
    # soff is a multiple of 4, so the residue r is 0 or 4; add 4 nodes x D=1.
    spp = GCHUNK // P  # slots per partition per chunk
    r = (spp - soff % spp) % spp
    if r:
        assert r % 4 == 0
        bucket_meta.append((r // 4, 4, joff, soff))
        joff += 4
        soff += r

    n = joff
    s_tot = soff
    ROWS = P * n
    R_TOT = NCORES * ROWS
    assert R_TOT % LANES == 0
    assert R_TOT // LANES <= 32767, "int16 gather index limit"

    gid = core_of.astype(np.int64) * ROWS + part_of.astype(np.int64) * n + jpos

    occ = np.zeros((NCORES, P, n), dtype=bool)
    occ[core_of, part_of, jpos] = True
    free = np.nonzero(~occ.reshape(-1))[0]
    assert len(free) > 0
    dummy_gid = int(free[0])

    order = np.argsort(dst, kind="stable")
    starts = np.zeros(N + 1, np.int64)
    np.cumsum(deg_in, out=starts[1:])
    rank = np.arange(E, dtype=np.int64) - starts[dst[order]]
    d_ord = dst[order]
    s_ord = src[order]

    gsrc = np.full((NCORES, P, s_tot), dummy_gid, dtype=np.int64)
    gsrc[core_of[d_ord], part_of[d_ord], slot0[d_ord] + rank] = gid[s_ord]

    # gather row index (int16) and lane value per slot (mask built on device)
    idx_row = (gsrc >> 3).astype(np.int16)          # [NCORES, P, s_tot]
    lanev = (gsrc & 7).astype(_bf16())              # [NCORES, P, s_tot]

    deg_in_t = np.zeros((NCORES, P, n), np.float32)
    deg_out_t = np.zeros((NCORES, P, n), np.float32)
    deg_in_t[core_of, part_of, jpos] = deg_in
    deg_out_t[core_of, part_of, jpos] = deg_out

    plan = _Plan()
    plan.N, plan.E = N, E
    plan.n, plan.s_tot, plan.ROWS, plan.R_TOT = n, s_tot, ROWS, R_TOT
    plan.bucket_meta = bucket_meta
    plan.core_of, plan.part_of, plan.jpos = core_of, part_of, jpos
    plan.idx_row, plan.lanev = idx_row, lanev
    plan.deg_in_t, plan.deg_out_t = deg_in_t, deg_out_t
    return plan


def _wrap_idx_chunks(idx_row_core, s_tot):
    """Per gather chunk: [128, spp] slot block -> int16 wrapped [128, GCHUNK//16].
    Gather position i (p=i%128, col=i//128) must map to slot (p, cs+col):
    flat list f[i] = idx_row[i % 128, cs + i // 128]."""
    spp = GCHUNK // P
    nch = s_tot // spp
    out = np.empty((P, nch, GCHUNK // 16), np.int16)
    for ch in range(nch):
        blk = idx_row_core[:, ch * spp:(ch + 1) * spp]     # [128, spp]
        flat = blk.T.reshape(-1)                           # i = col*128 + p
        w = flat.reshape(GCHUNK // 16, 16).T               # [16, GCHUNK//16]
        out[:, ch, :] = np.tile(w, (8, 1))
    return out                                             # [128, nch, GCHUNK//16]


# ------------------------------------------------------------------ builder --

def _raw_dma_gather(nc, out_ap, in_ap, idxs_ap, num_idxs, elem_size, elem_step,
                    queue_num=0):
    g = nc.gpsimd
    stride_bytes = elem_step * mybir.dt.size(in_ap.dtype)
    assert stride_bytes % 256 == 0
    _in_ap = g.lower_ap_dma(in_ap, for_custom_bir_dma=True)
    _idxs_ap = g.lower_ap(idxs_ap)
    _out_ap = g.lower_ap(out_ap)
    return g.add_instruction(
        mybir.InstDMAGatherAnt(
            name=nc.get_next_instruction_name(),
            ins=[*_in_ap, _idxs_ap, g.lower_val_access(g.to_reg(num_idxs))],
            outs=[_out_ap],
            transpose=False,
            num_idxs=num_idxs,
            elem_size=elem_size,
            stride_bytes_256=stride_bytes // 256,
            gen_mode=0,
            single_packet=True,
            queue_num=queue_num,
            sbuf_tokens_per_rank=0,
            sbuf_free_dim_per_rank=0,
            sbuf_free_dim_pad_per_rank=0,
            sbuf_byte_offset=0,
        )
    )


def _build(plan, F, H, C):
    n, s_tot, ROWS, R_TOT = plan.n, plan.s_tot, plan.ROWS, plan.R_TOT
    assert ROWS % NCOL == 0
    ncol_chunks = ROWS // NCOL
    spp = GCHUNK // P                 # slot columns per gather chunk
    nch = s_tot // spp                # gather chunks per step
    TROWS = R_TOT // LANES            # 256B table rows

    nc = bacc.Bacc("TRN2", target_bir_lowering=False, debug=False,
                   num_devices=NCORES, num_swdge_queues=NQ)

    ftT = nc.declare_dram_parameter("featT", [F, ROWS], fp32, isOutput=False)
    w0 = nc.declare_dram_parameter("W0", [F, H], fp32, isOutput=False)
    b0 = nc.declare_dram_parameter("b0", [H, 1], fp32, isOutput=False)
    w1 = nc.declare_dram_parameter("W1", [H, H], fp32, isOutput=False)
    b1 = nc.declare_dram_parameter("b1", [H, 1], fp32, isOutput=False)
    w2 = nc.declare_dram_parameter("W2", [H, C], fp32, isOutput=False)
    b2r = nc.declare_dram_parameter("b2r", [P, C], fp32, isOutput=False)
    idxp = nc.declare_dram_parameter("idx", [P, nch, GCHUNK // 16], i16,
                                     isOutput=False)
    lvp = nc.declare_dram_parameter("lanev", [P, s_tot], bf16, isOutput=False)
    iop = nc.declare_dram_parameter("iota8", [P, LANES], bf16, isOutput=False)
    degi = nc.declare_dram_parameter("degi", [P, n], fp32, isOutput=False)
    dego = nc.declare_dram_parameter("dego", [P, n], fp32, isOutput=False)
    outp = nc.declare_dram_parameter("out", [P, n, C], fp32, isOutput=True)

    cc_in = nc.dram_tensor("cc_in", [P, n, C], bf16)
    cc_out = nc.dram_tensor("cc_out", [TROWS, LANES * C], bf16)

    with tile.TileContext(nc) as tc:
        with tc.tile_pool(name="persist", bufs=1) as pers, \
             tc.tile_pool(name="work", bufs=3) as work, \
             tc.tile_pool(name="gbuf", bufs=2) as gbuf, \
             tc.tile_pool(name="selp", bufs=2) as selp, \
             tc.tile_pool(name="psum", bufs=2, space="PSUM") as psum, \
             tc.tile_pool(name="psum3", bufs=2, space="PSUM") as psum3p:

            w0a = pers.tile([P, H], bf16, tag="w0a")
            w0b = pers.tile([P, H], bf16, tag="w0b")
            w1t = pers.tile([P, H], bf16, tag="w1t")
            w2t = pers.tile([P, C], bf16, tag="w2t")
            b0t = pers.tile([H, 1], fp32, tag="b0t")
            b1t = pers.tile([H, 1], fp32, tag="b1t")
            b2t = pers.tile([P, C], fp32, tag="b2t")
            idx_sb = pers.tile([P, nch, GCHUNK // 16], i16, tag="idx")
            lv_sb = pers.tile([P, s_tot], bf16, tag="lanev")
            io_sb = pers.tile([P, LANES], bf16, tag="iota8")
            nsrc = pers.tile([P, n], fp32, tag="nsrc")
            ndst9 = pers.tile([P, n], fp32, tag="ndst9")
            h = pers.tile([P, n, C], fp32, tag="h")
            h0s = pers.tile([P, n, C], fp32, tag="h0s")
            agg = pers.tile([P, n, C], fp32, tag="agg")
            hs = pers.tile([P, n, C], bf16, tag="hs")
            msgs = pers.tile([P, s_tot, C], bf16, tag="msgs")
            h2T = pers.tile([P, ROWS], bf16, tag="h2T")

            for wt, src_ap in ((w0a, w0[0:P, :]), (w0b, w0[P:2 * P, :]),
                               (w1t, w1[:, :]), (w2t, w2[:, :])):
                tmp = work.tile(list(src_ap.shape), fp32, tag="wload")
                nc.sync.dma_start(out=tmp[:], in_=src_ap)
                nc.vector.tensor_copy(out=wt[:], in_=tmp[:])
            nc.sync.dma_start(out=b0t[:], in_=b0[:, :])
            nc.sync.dma_start(out=b1t[:], in_=b1[:, :])
            nc.sync.dma_start(out=b2t[:], in_=b2r[:, :])
            nc.sync.dma_start(out=idx_sb[:], in_=idxp[:, :, :])
            nc.sync.dma_start(out=lv_sb[:], in_=lvp[:, :])
            nc.sync.dma_start(out=io_sb[:], in_=iop[:, :])

            dtmp = work.tile([P, n], fp32, tag="deg")
            mask = work.tile([P, n], fp32, tag="mask")
            for deg_p, out_t, scale in ((dego, nsrc, 1.0),
                                        (degi, ndst9, 1.0 - ALPHA)):
                dsb = work.tile([P, n], fp32, tag="degload")
                nc.sync.dma_start(out=dsb[:], in_=deg_p[:, :])
                nc.vector.tensor_scalar(out=dtmp[:], in0=dsb[:], scalar1=1.0,
                                        scalar2=None, op0=mybir.AluOpType.max)
                nc.scalar.sqrt(out=dtmp[:], in_=dtmp[:])
                nc.vector.reciprocal(out=dtmp[:], in_=dtmp[:])
                nc.vector.tensor_scalar(out=mask[:], in0=dsb[:], scalar1=0.0,
                                        scalar2=None, op0=mybir.AluOpType.is_gt)
                if scale != 1.0:
                    nc.vector.tensor_scalar_mul(out=mask[:], in0=mask[:],
                                                scalar1=scale)
                nc.vector.tensor_tensor(out=out_t[:], in0=dtmp[:], in1=mask[:],
                                        op=mybir.AluOpType.mult)

            # ---- MLP
            for ch in range(ncol_chunks):
                cs = ch * NCOL
                xa = work.tile([P, NCOL], fp32, tag="xa")
                xb = work.tile([P, NCOL], fp32, tag="xb")
                nc.sync.dma_start(out=xa[:], in_=ftT[0:P, cs:cs + NCOL])
                nc.sync.dma_start(out=xb[:], in_=ftT[P:2 * P, cs:cs + NCOL])
                xab = work.tile([P, NCOL], bf16, tag="xab")
                xbb = work.tile([P, NCOL], bf16, tag="xbb")
                nc.vector.tensor_copy(out=xab[:], in_=xa[:])
                nc.vector.tensor_copy(out=xbb[:], in_=xb[:])
                ps1 = psum.tile([H, NCOL], fp32, tag="ps1")
                nc.tensor.matmul(ps1[:], w0a[:], xab[:], start=True, stop=False)
                nc.tensor.matmul(ps1[:], w0b[:], xbb[:], start=False, stop=True)
                h1 = work.tile([H, NCOL], bf16, tag="h1")
                nc.scalar.activation(out=h1[:], in_=ps1[:],
                                     func=mybir.ActivationFunctionType.Relu,
                                     bias=b0t[:, :1])
                ps2 = psum.tile([H, NCOL], fp32, tag="ps2")
                nc.tensor.matmul(ps2[:], w1t[:], h1[:], start=True, stop=True)
                nc.scalar.activation(out=h2T[:, cs:cs + NCOL], in_=ps2[:],
                                     func=mybir.ActivationFunctionType.Relu,
                                     bias=b1t[:, :1])
            for j in range(n):
                ps3 = psum3p.tile([P, C], fp32, tag="ps3")
                nc.tensor.matmul(ps3[:], h2T[:, j * P:(j + 1) * P], w2t[:],
                                 start=True, stop=True)
                nc.vector.tensor_tensor(out=h[:, j, :], in0=ps3[:], in1=b2t[:],
                                        op=mybir.AluOpType.add)

            nc.vector.tensor_scalar_mul(out=h0s[:], in0=h[:], scalar1=ALPHA)

            nsrc_b = nsrc[:].unsqueeze(2).to_broadcast([P, n, C])
            ndst9_b = ndst9[:].unsqueeze(2).to_broadcast([P, n, C])

            # ---- propagation
            for k in range(K_STEPS):
                nc.vector.tensor_tensor(out=hs[:], in0=h[:], in1=nsrc_b,
                                        op=mybir.AluOpType.mult)
                nc.sync.dma_start(out=cc_in[:, :, :], in_=hs[:])
                if not SKIP_CC:
                    nc.gpsimd.collective_compute(
                        "AllGather",
                        mybir.AluOpType.bypass,
                        replica_groups=[list(range(NCORES))],
                        ins=[cc_in.ap().opt()],
                        outs=[cc_out.ap().opt()],
                    )
                for ch in range(nch):
                    c0 = ch * spp
                    wide = gbuf.tile([P, spp, LANES * C], bf16, tag=f"wide{ch % NQ}")
                    if not SKIP_GATHER:
                        _raw_dma_gather(nc, wide[:, :, :], cc_out[:, :],
                                        idx_sb[:, ch, :], GCHUNK, LANES * C,
                                        LANES * C, queue_num=ch % NQ)
                    if SKIP_EXTRACT:
                        continue
                    # lane extract: wide [p, spp, LANES, C] * mask -> sum lanes
                    wv = wide[:].rearrange("p s (l c) -> p s l c", l=LANES)
                    mkc = selp.tile([P, spp, LANES], bf16, tag="mkc")
                    nc.vector.tensor_tensor(
                        out=mkc[:],
                        in0=lv_sb[:, c0:c0 + spp].unsqueeze(2).to_broadcast(
                            [P, spp, LANES]),
                        in1=io_sb[:].unsqueeze(1).to_broadcast([P, spp, LANES]),
                        op=mybir.AluOpType.is_equal)
                    mk = mkc[:].unsqueeze(3).to_broadcast([P, spp, LANES, C])
                    sel = selp.tile([P, spp, LANES, C], bf16, tag="sel")
                    nc.vector.tensor_tensor(out=sel[:], in0=wv, in1=mk,
                                            op=mybir.AluOpType.mult)
                    sv = sel[:].rearrange("p s l c -> p s c l")
                    # one-hot lane sum: 7 of 8 terms are exactly zero, so a
                    # bf16 destination loses nothing
                    with nc.allow_low_precision(reason="one-hot lane select"):
                        nc.vector.tensor_reduce(out=msgs[:, c0:c0 + spp, :],
                                                in_=sv,
                                                axis=mybir.AxisListType.X,
                                                op=mybir.AluOpType.add)
                for (D, m, joff, soff) in plan.bucket_meta:
                    view = msgs[:, soff:soff + m * D, :].rearrange(
                        "p (m d) c -> p m c d", m=m)
                    nc.vector.tensor_reduce(
                        out=agg[:, joff:joff + m, :], in_=view,
                        axis=mybir.AxisListType.X, op=mybir.AluOpType.add)
                nc.vector.tensor_tensor(out=h[:], in0=agg[:], in1=ndst9_b,
                                        op=mybir.AluOpType.mult)
                nc.vector.tensor_tensor(out=h[:], in0=h[:], in1=h0s[:],
                                        op=mybir.AluOpType.add)

            nc.sync.dma_start(out=outp[:, :, :], in_=h[:])

    nc.compile()
    return nc


# ------------------------------------------------------------------- kernel --

def kernel(features, W0, b0, W1, b1, W2, b2, src, dst):
    features = np.asarray(features, dtype=np.float32)
    W0 = np.asarray(W0, dtype=np.float32)
    b0 = np.asarray(b0, dtype=np.float32)
    W1 = np.asarray(W1, dtype=np.float32)
    b1 = np.asarray(b1, dtype=np.float32)
    W2 = np.asarray(W2, dtype=np.float32)
    b2 = np.asarray(b2, dtype=np.float32)
    src = np.asarray(src, dtype=np.int32)
    dst = np.asarray(dst, dtype=np.int32)

    N, F = features.shape
    H = W0.shape[1]
    C = W2.shape[1]

    plan = _make_plan(src, dst, N)
    nc = _build(plan, F, H, C)

    col = plan.jpos * P + plan.part_of
    b2rep = np.broadcast_to(b2.reshape(1, C), (P, C)).astype(np.float32).copy()
    in_maps = []
    for c in range(NCORES):
        sel = plan.core_of == c
        ft = np.zeros((F, plan.ROWS), np.float32)
        ft[:, col[sel]] = features[sel].T
        in_maps.append({
            "featT": ft,
            "W0": W0, "b0": b0.reshape(H, 1),
            "W1": W1, "b1": b1.reshape(H, 1),
            "W2": W2, "b2r": b2rep,
            "idx": _wrap_idx_chunks(plan.idx_row[c], plan.s_tot),
            "lanev": plan.lanev[c],
            "iota8": _IOTA8,
            "degi": plan.deg_in_t[c],
            "dego": plan.deg_out_t[c],
        })

    res = run_bass_kernel_spmd(nc, in_maps, core_ids=list(range(NCORES)),
                               trace=False)

    arr = np.stack([res.results[c]["out"] for c in range(NCORES)])
    arr = arr.reshape(NCORES, P, plan.n, C)
    out = np.empty((N, C), np.float32)
    out[:] = arr[plan.core_of, plan.part_of, plan.jpos]
    return out
